# revision 1
# baseline (speedup 1.0000x reference)
"""GCN layer (gather + scatter-add message passing) on 8 Trainium2 NeuronCores.

Strategy (dst-partitioned node sharding, per the sharding hint):
  - Node blocks of 128 assigned to (core, slot) by balanced bin-packing
    (blocks sorted by edge count; slot g gets ranked blocks [8g, 8g+8)), so
    the shared per-slot tile counts T_LO/T_HI carry minimal padding.
  - Host sorts each block's non-self edges by src-table-half and pads each
    (block, half) group to a multiple of 128.  Self-loops never enter the
    edge stream: their contribution dinv[n]^2 * (x[n] @ W) is a separate
    per-block GEMM from a sequentially-loaded x^T slice.
  - Device phase per 128-edge tile: dma_gather 256B fp16 x rows by src
    (GPSIMD/SWDGE), then matmul-accumulate agg[f, d] += msg[e, f]^T @ S[e, d]
    where S is the host-built one-hot selection matrix pre-scaled by
    dinv[src] (streamed from HBM over HWDGE; no vector-engine work per edge).
  - Per block: gem = (agg^T @ W16) [d, f2] plus gem2 = (xT_blk @ W16) for the
    self loop; out = dinv_d*gem + dinv_d^2*gem2 + b, DMA'd to the core's
    output slice.  (The linear layer commutes with the aggregation, so the
    GEMM runs on aggregated rows, not all edges.)

The edge tables are padded so the instruction stream is identical on all 8
cores (run_bass_kernel_spmd compiles one program); only tensor data differs.
"""

import sys

sys.path.insert(0, "/opt/trn_rl_repo")

import numpy as np

import concourse.bass as bass
import concourse.bacc as bacc
import concourse.mybir as mybir
import concourse.tile as tile
import concourse.tile_sem_assignment as _tsa
from concourse.tile import add_dep_helper

# Tile round-robins SWDGE DMAs over the 8 DMASW sem lanes in scheduling
# order, which lets one sem serve instructions on different SWDGE queues.
# The ucode's per-queue ring reclaim then sees foreign increments (CoreSim
# flags this as "sem locked to SWDGE queue").  Pin lanes per queue instead:
# queue q only ever uses lanes {q, q+4}.
if not getattr(_tsa.TileClockTick, "_gcn_queue_aware", False):
    _orig_assign_tick = _tsa.TileClockTick._assign_tick

    def _assign_tick_queue_aware(self, inst):
        if (
            isinstance(inst, _tsa.DMAInst)
            and inst.engine == mybir.EngineType.Pool
            and not isinstance(inst, _tsa.bass_isa.UserSyncedRemoteDMADescs)
            and self.swdge_sem_count == _tsa.NUM_SWDGE_GLOBAL_SEMS
        ):
            q = getattr(inst, "queue_num", 0) or 0
            toggles = getattr(self, "_gcn_q_toggle", None)
            if toggles is None:
                toggles = self._gcn_q_toggle = [0, 0, 0, 0]
            self.next_sw_dma_idx = q + 4 * toggles[q]
            toggles[q] ^= 1
        return _orig_assign_tick(self, inst)

    _tsa.TileClockTick._assign_tick = _assign_tick_queue_aware
    _tsa.TileClockTick._gcn_queue_aware = True

N = 50000
E = 800000
F = 128          # in/out channels
P = 128
NCORES = 8
NB = 392         # node blocks incl. padding (= 8 * 49)
G = NB // NCORES  # 49 slots per core
LO = 32768       # gather-table split (int16 index limit)
NPAD = 51200     # padded node rows

f32 = mybir.dt.float32
fp16 = mybir.dt.float16
i32 = mybir.dt.int32
i16 = mybir.dt.int16

# Gathers merge GGRP slots per call: Tile has only 8 SWDGE sem lanes
# (2 per queue), so at most 8 gather DMAs are in flight; bigger calls
# put more descriptors in flight per sem slot.
GGRP = 2


def _gather_order(T_LO, T_HI):
    """(side, slot) issue order shared by host packing and device build."""
    order = []
    for g in range(G):
        if T_LO[g]:
            order.append(("lo", g))
        if T_HI[g]:
            order.append(("hi", g))
    return order


def _host_prep(x, W, b, edge_index):
    """Index manipulation + data staging (incl. host-computed norm coeffs)."""
    x = np.asarray(x, dtype=np.float32)
    W = np.asarray(W, dtype=np.float32)
    b = np.asarray(b, dtype=np.float32)
    ei = np.asarray(edge_index)
    src = ei[0].astype(np.int64)
    dst = ei[1].astype(np.int64)

    cnt = np.bincount(dst, minlength=NPAD).astype(np.int64)
    # GCN symmetric normalization; deg includes the self loop -> cnt + 1.
    dinv = (1.0 / np.sqrt(cnt.astype(np.float64) + 1.0)).astype(np.float32)
    dinv16 = dinv.astype(np.float16)

    # Sort edges by (dst block, src table half, src).  The src subsort makes
    # each gather's descriptors walk HBM in ascending address order.
    ishi = (src >= LO).astype(np.int64)
    blk = dst >> 7
    order = np.lexsort((src, ishi, blk))
    src_s, dst_s, ishi_s, blk_s = src[order], dst[order], ishi[order], blk[order]
    bounds = np.searchsorted(blk_s, np.arange(NB + 1))

    # Per-block edge lists split by table half.
    blo_idx, blo_dst, bhi_idx, bhi_dst = [], [], [], []
    lo_tiles = np.zeros(NB, np.int64)
    hi_tiles = np.zeros(NB, np.int64)
    for bb_ in range(NB):
        s0, s1 = bounds[bb_], bounds[bb_ + 1]
        mid = s0 + int(np.searchsorted(ishi_s[s0:s1], 1))
        blo_idx.append(src_s[s0:mid])
        blo_dst.append(dst_s[s0:mid] - 128 * bb_)
        bhi_idx.append(src_s[mid:s1] - LO)
        bhi_dst.append(dst_s[mid:s1] - 128 * bb_)
        lo_tiles[bb_] = -(-(mid - s0) // 128)
        hi_tiles[bb_] = -(-(s1 - mid) // 128)

    # Balanced assignment: blocks ranked by total tile need; slot g gets
    # ranks [8g, 8g+8) so the per-slot max over cores stays near the mean.
    rank = np.argsort(-(lo_tiles + hi_tiles), kind="stable")
    blk_of = [[0] * G for _ in range(NCORES)]
    for g in range(G):
        for c in range(NCORES):
            blk_of[c][g] = int(rank[8 * g + c])

    T_LO = [max(1, max(lo_tiles[blk_of[c][g]] for c in range(NCORES)))
            for g in range(G)]
    T_HI = [max(hi_tiles[blk_of[c][g]] for c in range(NCORES))
            for g in range(G)]
    T_LO = [int(t) for t in T_LO]
    T_HI = [int(t) for t in T_HI]
    NT = sum(T_LO) + sum(T_HI)
    LTOT = NT * 8

    # fp16 staging of x (gather table) and its transpose (self-loop GEMM
    # stationary).  One extra rounding vs fp32; numerically fine at 2e-2.
    x16 = x.astype(np.float16)
    x_pad = np.zeros((NPAD, F), np.float16)
    x_pad[:N] = x16
    xT_pad = np.zeros((F, NPAD), np.float16)
    xT_pad[:, :N] = x16.T

    bb_host = np.tile(b[None, :], (P, 1)).astype(np.float32)

    in_maps = []
    for c in range(NCORES):
        Svals = np.zeros((P, NT * P), np.float16)
        Sv = Svals.reshape(P, NT, P)
        idx16 = np.zeros((P, LTOT), np.int16)
        dinv_slot = np.zeros((P, G), np.float32)
        xtsel = np.zeros((F, G * P), np.float16)
        for g in range(G):
            bb_ = blk_of[c][g]
            xtsel[:, g * P : (g + 1) * P] = xT_pad[:, 128 * bb_ : 128 * bb_ + P]

        def side_data(side, g):
            bb_ = blk_of[c][g]
            if side == "lo":
                return T_LO[g], blo_idx[bb_], blo_dst[bb_], 0
            return T_HI[g], bhi_idx[bb_], bhi_dst[bb_], LO

        # S values: per-slot column order (lo tiles then hi tiles)
        col = 0
        for side, g in _gather_order(T_LO, T_HI):
            nt, li, ld, glob_off = side_data(side, g)
            ne = len(li)
            # scatter the pre-scaled one-hot: edge j -> partition j%128,
            # tile j//128, column ld[j]
            j = np.arange(ne)
            Sv[j % 128, col + j // 128, ld] = dinv16[li + glob_off]
            col += nt
        # gather indices: grouped order (all lo of the slot group, then
        # all hi), matching the device's merged gather calls
        icol = 0
        for k in range(-(-G // GGRP)):
            gs = range(k * GGRP, min((k + 1) * GGRP, G))
            for side in ("lo", "hi"):
                for g in gs:
                    nt, li, ld, _ = side_data(side, g)
                    ne = len(li)
                    pi = np.zeros(nt * 128, np.int64)
                    pi[:ne] = li
                    k8 = nt * 8
                    idx16[:, icol : icol + k8] = np.tile(
                        pi.reshape(-1, 16).T.astype(np.int16), (8, 1)
                    )
                    icol += k8
        for g in range(G):
            bb_ = blk_of[c][g]
            dinv_slot[:, g] = dinv[128 * bb_ : 128 * (bb_ + 1)]
        in_maps.append(
            {
                "x": x_pad,
                "xt": xtsel,
                "w": W.astype(np.float16),
                "bb": bb_host,
                "sval": Svals,
                "idx16": idx16,
                "dinv_slot": dinv_slot,
                "dinv2_slot": dinv_slot * dinv_slot,
            }
        )
    return in_maps, T_LO, T_HI, blk_of


def build_nc(T_LO, T_HI, blk_of, debug=False):
    NT = sum(T_LO) + sum(T_HI)
    LTOT = NT * 8
    nc = bacc.Bacc(
        "TRN2",
        target_bir_lowering=False,
        debug=debug,
        num_swdge_queues=4,
        # default 16KB/partition caps the SWDGE descriptor rings at ~2-3
        # outstanding gathers per queue; the Pool engine then stalls in-order
        # and the whole pipeline runs burst-idle-burst.  3x rings -> real
        # gather lookahead.
        dynamic_dma_scratch_size=49152,
    )

    x_d = nc.dram_tensor("x", [NPAD, F], fp16, kind="ExternalInput")
    xt_d = nc.dram_tensor("xt", [F, G * P], fp16, kind="ExternalInput")
    w_d = nc.dram_tensor("w", [F, F], fp16, kind="ExternalInput")
    bb_d = nc.dram_tensor("bb", [P, F], f32, kind="ExternalInput")
    sval_d = nc.dram_tensor("sval", [P, NT * P], fp16, kind="ExternalInput")
    idx_d = nc.dram_tensor("idx16", [P, LTOT], i16, kind="ExternalInput")
    dinv_d = nc.dram_tensor("dinv_slot", [P, G], f32, kind="ExternalInput")
    dinv2_d = nc.dram_tensor("dinv2_slot", [P, G], f32, kind="ExternalInput")
    out_d = nc.dram_tensor("out", [G * P, F], f32, kind="ExternalOutput")

    with tile.TileContext(nc) as tc:
        with (
            tc.tile_pool(name="const", bufs=1) as cp,
            tc.tile_pool(name="mlo", bufs=6) as plo,
            tc.tile_pool(name="mhi", bufs=6) as phi,
            tc.tile_pool(name="sel", bufs=6) as psel,
            tc.tile_pool(name="xtb", bufs=3) as pxt,
            tc.tile_pool(name="tt", bufs=3) as ptt,
            tc.tile_pool(name="osb", bufs=3) as posb,
            tc.tile_pool(name="agg", bufs=3, space="PSUM") as pagg,
            tc.tile_pool(name="gem", bufs=2, space="PSUM") as pgem,
            tc.tile_pool(name="gem2", bufs=2, space="PSUM") as pgem2,
        ):
            w_sb = cp.tile([F, F], fp16)
            nc.sync.dma_start(out=w_sb[:], in_=w_d[:])
            bb_sb = cp.tile([P, F], f32)
            nc.sync.dma_start(out=bb_sb[:], in_=bb_d[:])
            idx_sb = cp.tile([P, LTOT], i16)
            nc.sync.dma_start(out=idx_sb[:], in_=idx_d[:])
            dinv_sb = cp.tile([P, G], f32)
            nc.sync.dma_start(out=dinv_sb[:], in_=dinv_d[:])
            dinv2_sb = cp.tile([P, G], f32)
            nc.sync.dma_start(out=dinv2_sb[:], in_=dinv2_d[:])

            # ---- per-edge phase ----
            # Gather raw fp16 x rows by src (SWDGE); stream the pre-scaled
            # one-hot S from HBM (HWDGE).  One gather per (slot, table-half):
            # independent destination tiles let the 4 SWDGE queues generate
            # descriptors concurrently.
            lo_tab = x_d[0:LO, :]
            hi_tab = x_d[LO:NPAD, :]
            icol = [0]
            qrr = [0]

            def gather(pool, tag, tab, nt):
                m = pool.tile([P, nt * F], fp16, tag=tag)
                nc.gpsimd.dma_gather(
                    out_ap=m[:].rearrange("p (k f) -> p k f", f=F),
                    in_ap=tab,
                    idxs_ap=idx_sb[:, icol[0] : icol[0] + nt * 8],
                    num_idxs=nt * P,
                    num_idxs_reg=nt * P,
                    elem_size=F,
                    single_packet=False,
                    queue_num=qrr[0] % 4,
                )
                qrr[0] += 1
                icol[0] += nt * 8
                return m

            # Gather-call merging: slots [k*GGRP, (k+1)*GGRP) share one lo
            # gather and one hi gather (consumers slice the merged tile).
            # Column bookkeeping follows the host packing order (per slot:
            # lo tiles then hi tiles).
            cols = {}
            cc = 0
            for g in range(G):
                cols[("lo", g)] = cc
                cc += T_LO[g]
                cols[("hi", g)] = cc
                cc += T_HI[g]

            ngrp = -(-G // GGRP)
            grp_lo = [None] * ngrp
            grp_hi = [None] * ngrp
            # within-group tile offset of each slot's lo/hi section
            off_lo = {}
            off_hi = {}
            for k in range(ngrp):
                gs = list(range(k * GGRP, min((k + 1) * GGRP, G)))
                ol = 0
                for g in gs:
                    off_lo[g] = ol
                    ol += T_LO[g]
                oh = 0
                for g in gs:
                    off_hi[g] = oh
                    oh += T_HI[g]

            def issue_group(k):
                gs = list(range(k * GGRP, min((k + 1) * GGRP, G)))
                ntl = sum(T_LO[g] for g in gs)
                nth = sum(T_HI[g] for g in gs)
                if ntl:
                    grp_lo[k] = gather(plo, "mlo", lo_tab, ntl)
                if nth:
                    grp_hi[k] = gather(phi, "mhi", hi_tab, nth)

            # NOTE: idx16 host packing must match this issue order: all lo
            # indices of the group first, then all hi indices.
            next_issue = [0]

            def ensure_issued(upto):
                while next_issue[0] <= min(upto, ngrp - 1):
                    issue_group(next_issue[0])
                    next_issue[0] += 1

            S_slot = [None] * G

            def fetch_S(g):
                ntot = T_LO[g] + T_HI[g]
                S = psel.tile([P, ntot * P], fp16, tag="S")
                base = cols[("lo", g)]
                nc.sync.dma_start(
                    out=S[:], in_=sval_d[:, base * P : (base + ntot) * P]
                )
                S_slot[g] = S

            for g in range(G):
                k = g // GGRP
                ensure_issued(k + 2)
                for gg in range(g, min(g + 4, G)):
                    if S_slot[gg] is None:
                        fetch_S(gg)
                ntl, nth = T_LO[g], T_HI[g]
                ntot = ntl + nth
                S = S_slot[g]

                agg = pagg.tile([P, P], f32, tag="agg")
                mm = 0
                for (nt, m, toff, scol) in (
                    (ntl, grp_lo[k], off_lo[g], 0),
                    (nth, grp_hi[k], off_hi[g], ntl),
                ):
                    if nt == 0:
                        continue
                    for t in range(nt):
                        nc.tensor.matmul(
                            out=agg[:],
                            lhsT=m[:, (toff + t) * F : (toff + t + 1) * F],
                            rhs=S[:, (scol + t) * P : (scol + t + 1) * P],
                            start=(mm == 0),
                            stop=(mm == ntot - 1),
                        )
                        mm += 1

                # self-loop stationary: x^T columns for this core's slot-g
                # block (host packs the per-core block into slot order)
                xt_sb = pxt.tile([F, P], fp16, tag="xt")
                nc.sync.dma_start(
                    out=xt_sb[:], in_=xt_d[:, g * P : (g + 1) * P]
                )

                tt = ptt.tile([P, P], fp16, tag="tt")
                nc.scalar.activation(
                    out=tt[:], in_=agg[:],
                    func=mybir.ActivationFunctionType.Copy,
                )
                gem = pgem.tile([P, P], f32, tag="gem")
                nc.tensor.matmul(
                    out=gem[:], lhsT=tt[:], rhs=w_sb[:], start=True, stop=True
                )
                gem2 = pgem2.tile([P, P], f32, tag="gem2")
                nc.tensor.matmul(
                    out=gem2[:], lhsT=xt_sb[:], rhs=w_sb[:], start=True,
                    stop=True,
                )
                # out = dinv*gem + dinv^2*gem2 + b
                t2 = posb.tile([P, P], f32, tag="t2")
                nc.scalar.activation(
                    out=t2[:], in_=gem2[:],
                    func=mybir.ActivationFunctionType.Copy,
                    scale=dinv2_sb[:, g : g + 1],
                )
                osb = posb.tile([P, P], f32, tag="osb")
                nc.vector.tensor_scalar(
                    out=osb[:], in0=gem[:],
                    scalar1=dinv_sb[:, g : g + 1], scalar2=None,
                    op0=mybir.AluOpType.mult,
                )
                nc.vector.tensor_tensor(
                    out=osb[:], in0=osb[:], in1=t2[:],
                    op=mybir.AluOpType.add,
                )
                nc.vector.tensor_tensor(
                    out=osb[:], in0=osb[:], in1=bb_sb[:],
                    op=mybir.AluOpType.add,
                )
                nc.sync.dma_start(
                    out=out_d[g * P : (g + 1) * P, :], in_=osb[:]
                )

    nc.compile()
    return nc


def _assemble(results, blk_of):
    out = np.zeros((NB * P, F), np.float32)
    for c in range(NCORES):
        oc = results[c]["out"]
        for g in range(G):
            bb_ = blk_of[c][g]
            out[bb_ * P : (bb_ + 1) * P] = oc[g * P : (g + 1) * P]
    return out[:N]


def kernel(x, W, b, edge_index):
    from concourse.bass_utils import run_bass_kernel_spmd

    in_maps, T_LO, T_HI, blk_of = _host_prep(x, W, b, edge_index)
    nc = build_nc(T_LO, T_HI, blk_of)
    res = run_bass_kernel_spmd(nc, in_maps, list(range(NCORES)))
    return _assemble(res.results, blk_of)

